# revision 1
# baseline (speedup 1.0000x reference)
"""Trainium2 kernel for nn_MultiHeadCrossAttention_28063316313030.

Math: with seq_len == 1, softmax over a size-1 axis is identically 1, so
attention(Q,K,V) == V and W_Q/W_K are dead code.  The whole module collapses to

    out = LN(x1 @ A) + LN(x2 @ A),   A = W_V.T @ W_fc.T   (1024 x 1024)

where LN is LayerNorm over the last dim with gamma/beta.

Distribution: pure data parallel over the batch dim across 8 NeuronCores.
Host precomputes A (tiny matmul) and pre-tiles x1/x2 C-major so the TensorE
contraction dim lands on SBUF partitions with fully contiguous DMA runs.

Device per core (2048 rows per stream):
  PE warmup (full-duty dummy matmuls) opens the HAM clock gate early.
  b-tiles are processed in PAIRS, k-major across 4 (tile,stream) groups into
  8 PSUM banks: each 256KB A chunk feeds 8 matmuls, so the PE stays dense
  even while A is still streaming in from HBM at kernel start.
  Per group: z = xT.T @ A (f32r), bn_stats/bn_aggr -> mean/var,
  r = 1/sqrt(var+eps) (ACT sqrt + DVE recip), n = z*r - mu*r (ACT Identity,
  per-partition scale/bias).  out_tile = n1 + n2 (DVE), optional affine, DMA.
"""

import sys

sys.path.insert(0, "/opt/trn_rl_repo")

import numpy as np

B, C, OUT = 16384, 1024, 1024
EPS = 1e-5
NCORES = 8
R = B // NCORES  # rows per core per stream
P = 128
KT = C // P  # contraction tiles
BT = R // P  # row tiles per core
NH = OUT // 512  # psum bank halves per row tile
N_WARMUP = 8

_cache = {}


def _build(use_affine: bool, mm_dtype_name: str):
    import concourse.bacc as bacc
    import concourse.bass as bass
    import concourse.mybir as mybir
    from concourse.tile import TileContext

    f32 = mybir.dt.float32
    mmdt = getattr(mybir.dt, mm_dtype_name)
    AF = mybir.ActivationFunctionType
    ALU = mybir.AluOpType

    nc = bacc.Bacc("TRN2", target_bir_lowering=False, debug=False, num_devices=NCORES)

    # host-pretiled: [ki, bt, ko, bi]
    x1p = nc.declare_dram_parameter("x1p", [P, BT, KT, P], mmdt, isOutput=False)
    x2p = nc.declare_dram_parameter("x2p", [P, BT, KT, P], mmdt, isOutput=False)
    # host-pretiled: [ki, ko, o]
    a_d = nc.declare_dram_parameter("a", [P, KT, OUT], mmdt, isOutput=False)
    if use_affine:
        gam_d = nc.declare_dram_parameter("gamma", [OUT], f32, isOutput=False)
        bet2_d = nc.declare_dram_parameter("beta2", [OUT], f32, isOutput=False)
    y_d = nc.declare_dram_parameter("y", [R, OUT], f32, isOutput=True)

    with TileContext(nc) as tc:
        with (
            tc.tile_pool(name="singles", bufs=1) as singles,
            tc.tile_pool(name="xs", bufs=3) as xpool,
            tc.tile_pool(name="ns", bufs=3) as npool,
            tc.tile_pool(name="outs", bufs=3) as opool,
            tc.tile_pool(name="stats", bufs=4) as stats,
            tc.tile_pool(name="psum", bufs=2, space="PSUM") as psum,
        ):
            def psum_tag(bt, s, h):
                return f"ps{s}{h}"

            # --- warm tiles land first on the ring; PE warmup matmuls trip
            # the HAM activity window so the clock gate opens before the
            # real matmuls arrive.  Alternating two stationaries lets each
            # LDWEIGHTS overlap the previous matmul (background weight
            # buffer), keeping the duty cycle near 100%.
            bf16 = mybir.dt.bfloat16
            warm_sb = singles.tile([P, 512], bf16)
            nc.sync.dma_start(warm_sb[:], a_d[:, 0, 0:256].bitcast(bf16))
            warm_ps = psum.tile([P, 512], f32, tag=psum_tag(1, 1, 1))
            for w in range(N_WARMUP):
                lo = 128 * (w % 2)
                nc.tensor.matmul(
                    warm_ps[:], lhsT=warm_sb[:, lo : lo + P], rhs=warm_sb[:],
                    start=True, stop=True,
                )

            # --- ring order for the startup race: bt=0 x tiles, then A in
            # consumption order.
            xt_pre = {}
            for s, xp in enumerate((x1p, x2p)):
                t = xpool.tile([P, KT, P], mmdt, tag=f"xt{s}", name=f"xt_pre_{s}")
                nc.sync.dma_start(t[:], xp[:, 0])
                xt_pre[(0, s)] = t

            a_sb = [[None] * NH for _ in range(KT)]
            for k in range(KT):
                for h in range(NH):
                    t = singles.tile([P, 512], mmdt, tag=f"a{k}_{h}", name=f"a{k}_{h}")
                    nc.sync.dma_start(t[:], a_d[:, k, h * 512 : (h + 1) * 512])
                    a_sb[k][h] = t

            eps_sb = singles.tile([P, 1], f32)
            nc.vector.memset(eps_sb, EPS)
            if use_affine:
                gam_sb = singles.tile([P, OUT], f32)
                nc.sync.dma_start(
                    gam_sb[:],
                    bass.AP(
                        tensor=gam_d.tensor,
                        offset=gam_d.offset,
                        ap=[[0, P], gam_d.ap[0]],
                    ),
                )
                bet2_sb = singles.tile([P, OUT], f32)
                nc.sync.dma_start(
                    bet2_sb[:],
                    bass.AP(
                        tensor=bet2_d.tensor,
                        offset=bet2_d.offset,
                        ap=[[0, P], bet2_d.ap[0]],
                    ),
                )

            def epilogue(bt, s, ps_tiles):
                """LayerNorm the two PSUM halves of group (bt, s) -> SBUF."""
                st = stats.tile([P, NH, 6], f32, tag=f"st{s}", name=f"st{bt}{s}")
                for h in range(NH):
                    nc.vector.bn_stats(st[:, h, :], ps_tiles[h][:])
                mv = stats.tile([P, 2], f32, tag=f"mv{s}", name=f"mv{bt}{s}")
                nc.vector.bn_aggr(mv[:], st[:])

                r_sb = stats.tile([P, 1], f32, tag=f"r{s}", name=f"r{bt}{s}")
                nc.scalar.activation(
                    r_sb[:], mv[:, 1:2], func=AF.Sqrt, bias=eps_sb[:], scale=1.0
                )
                nc.vector.reciprocal(r_sb[:], r_sb[:])
                nmr = stats.tile([P, 1], f32, tag=f"nmr{s}", name=f"nmr{bt}{s}")
                nc.vector.tensor_scalar(
                    nmr[:],
                    mv[:, 0:1],
                    scalar1=r_sb[:],
                    scalar2=-1.0,
                    op0=ALU.mult,
                    op1=ALU.mult,
                )

                ntile = npool.tile([P, OUT], f32, tag=f"n{s}", name=f"n{bt}{s}")
                for h in range(NH):
                    nc.scalar.activation(
                        ntile[:, h * 512 : (h + 1) * 512],
                        ps_tiles[h][:],
                        func=AF.Identity,
                        bias=nmr[:],
                        scale=r_sb[:],
                    )
                return ntile

            def store(bt, n_pair):
                out_t = opool.tile([P, OUT], f32, tag="out", name=f"out{bt}")
                for h in range(NH):
                    sl = slice(h * 512, (h + 1) * 512)
                    nc.vector.tensor_tensor(
                        out_t[:, sl], n_pair[0][:, sl], n_pair[1][:, sl],
                        op=ALU.add,
                    )
                    if use_affine:
                        nc.vector.tensor_tensor(
                            out_t[:, sl], out_t[:, sl], gam_sb[:, sl], op=ALU.mult
                        )
                        nc.vector.tensor_tensor(
                            out_t[:, sl], out_t[:, sl], bet2_sb[:, sl], op=ALU.add
                        )
                    nc.sync.dma_start(
                        y_d[bt * P : (bt + 1) * P, sl], out_t[:, sl]
                    )

            for bt in range(BT):
                xts = {}
                for s in range(2):
                    if (bt, s) in xt_pre:
                        xts[s] = xt_pre[(bt, s)]
                    else:
                        xt = xpool.tile(
                            [P, KT, P], mmdt, tag=f"xt{s}", name=f"xt{bt}_{s}"
                        )
                        nc.sync.dma_start(xt[:], (x1p, x2p)[s][:, bt])
                        xts[s] = xt

                ps = {
                    s: [
                        psum.tile(
                            [P, 512], f32, tag=psum_tag(bt, s, h),
                            name=f"ps{bt}{s}{h}",
                        )
                        for h in range(NH)
                    ]
                    for s in range(2)
                }

                if bt < BT - 1:
                    # k-major across both streams: 4 matmuls per A chunk
                    # pair, so the PE keeps pace with the A DMA stream at
                    # kernel start.
                    for k in range(KT):
                        for s in range(2):
                            for h in range(NH):
                                nc.tensor.matmul(
                                    ps[s][h][:],
                                    lhsT=xts[s][:, k, :],
                                    rhs=a_sb[k][h][:],
                                    start=(k == 0),
                                    stop=(k == KT - 1),
                                )
                    n_pair = [epilogue(bt, s, ps[s]) for s in range(2)]
                    store(bt, n_pair)
                else:
                    # Tail: serialize the streams so epilogues stagger; the
                    # very last stream runs h-outer so its h0 stats overlap
                    # its h1 matmuls.
                    n_pair = []
                    for s in range(2):
                        order = (
                            [(h, k) for h in range(NH) for k in range(KT)]
                            if s == 1
                            else [(h, k) for k in range(KT) for h in range(NH)]
                        )
                        for h, k in order:
                            nc.tensor.matmul(
                                ps[s][h][:],
                                lhsT=xts[s][:, k, :],
                                rhs=a_sb[k][h][:],
                                start=(k == 0),
                                stop=(k == KT - 1),
                            )
                        n_pair.append(epilogue(bt, s, ps[s]))
                    store(bt, n_pair)

    nc.finalize()
    return nc


def _get_nc(use_affine: bool, mm_dtype_name: str):
    key = (use_affine, mm_dtype_name)
    if key not in _cache:
        _cache[key] = _build(use_affine, mm_dtype_name)
    return _cache[key]


def _pretile_x(x_core: np.ndarray) -> np.ndarray:
    # [R, C] -> [ki, bt, ko, bi]
    return np.ascontiguousarray(
        x_core.reshape(BT, P, KT, P).transpose(3, 0, 2, 1)
    )


def kernel(x1, x2, W_Q, W_K, W_V, W_fc, gamma, beta, _trace=False,
           _mm_dtype="float32r"):
    from concourse.bass_utils import run_bass_kernel_spmd

    x1 = np.asarray(x1, dtype=np.float32)
    x2 = np.asarray(x2, dtype=np.float32)
    W_V = np.asarray(W_V, dtype=np.float32)
    W_fc = np.asarray(W_fc, dtype=np.float32)
    gamma = np.asarray(gamma, dtype=np.float32)
    beta = np.asarray(beta, dtype=np.float32)

    # A = W_V.T @ W_fc.T in float64 to keep the host collapse error negligible.
    A = (W_V.T.astype(np.float64) @ W_fc.T.astype(np.float64)).astype(np.float32)
    # [C, OUT] -> [ki, ko, o]
    Ap = np.ascontiguousarray(A.reshape(KT, P, OUT).transpose(1, 0, 2))

    use_affine = not (np.all(gamma == 1.0) and np.all(beta == 0.0))

    if _mm_dtype == "bfloat16":
        import ml_dtypes

        np_mm = ml_dtypes.bfloat16
    else:
        np_mm = np.float32
    Ap = Ap.astype(np_mm)

    in_maps = []
    for r in range(NCORES):
        sl = slice(r * R, (r + 1) * R)
        m = {
            "x1p": _pretile_x(x1[sl]).astype(np_mm),
            "x2p": _pretile_x(x2[sl]).astype(np_mm),
            "a": Ap,
        }
        if use_affine:
            m["gamma"] = gamma
            m["beta2"] = (2.0 * beta).astype(np.float32)
        in_maps.append(m)

    nc = _get_nc(use_affine, _mm_dtype)
    res = run_bass_kernel_spmd(nc, in_maps, list(range(NCORES)), trace=_trace)

    y = np.concatenate([res.results[r]["y"] for r in range(NCORES)], axis=0)
    out = y.reshape(B, 1, OUT)
    if _trace:
        return out, res
    return out



# revision 5
# speedup vs baseline: 1.0580x; 1.0580x over previous
"""Trainium2 kernel for nn_MultiHeadCrossAttention_28063316313030.

Math: with seq_len == 1, softmax over a size-1 axis is identically 1, so
attention(Q,K,V) == V and W_Q/W_K are dead code.  The whole module collapses to

    out = LN(x1 @ A) + LN(x2 @ A),   A = W_V.T @ W_fc.T   (1024 x 1024)

where LN is LayerNorm over the last dim with gamma/beta.

Distribution: pure data parallel over the batch dim across 8 NeuronCores.
Host precomputes A (tiny matmul) and pre-tiles x1/x2 C-major so the TensorE
contraction dim lands on SBUF partitions with fully contiguous DMA runs.

Everything on the PE path is bf16 (x, A, and the stored output, which the
host upcasts to f32): the moving operand streams at ~218ns per 512-row
matmul vs ~233ns for fp32r, and DMA bytes halve.  LayerNorm stays in f32
(PSUM + stats).  Measured rel err ~4e-3 vs the 2e-2 gate.

Device per core (2048 rows per stream):
  PE warmup matmuls run on a memset tile (no DMA dependency) so Tensor
  activity starts the HAM/clock ramp at the first possible instant.
  DMA issue is spread across engine queues: A per-k tiles [128,1024] on
  GpSimd (2KB runs, 8 descriptors), x tiles and outputs on Sync.
  b-tiles are processed in PAIRS, k-major across 4 (tile,stream) groups into
  8 PSUM banks: each A k-tile feeds 8 matmuls, so the PE stays dense even
  while A is still streaming in from HBM at kernel start.
  Per group: z = xT.T @ A, bn_stats/bn_aggr -> mean/var,
  r = 1/sqrt(var+eps) (ACT sqrt + DVE recip), n = z*r - mu*r (ACT Identity,
  per-partition scale/bias).  out_tile = n1 + n2 (DVE, bf16 out), DMA.
"""

import sys

sys.path.insert(0, "/opt/trn_rl_repo")

import numpy as np

B, C, OUT = 16384, 1024, 1024
EPS = 1e-5
NCORES = 8
R = B // NCORES  # rows per core per stream
P = 128
KT = C // P  # contraction tiles
BT = R // P  # row tiles per core
NH = OUT // 512  # psum bank halves per row tile
N_WARMUP = 8

_cache = {}


def _build(mm_dtype_name: str, out_dtype_name: str):
    import concourse.bacc as bacc
    import concourse.bass as bass
    import concourse.mybir as mybir
    from concourse.tile import TileContext

    f32 = mybir.dt.float32
    bf16 = mybir.dt.bfloat16
    mmdt = getattr(mybir.dt, mm_dtype_name)
    outdt = getattr(mybir.dt, out_dtype_name)
    AF = mybir.ActivationFunctionType
    ALU = mybir.AluOpType

    nc = bacc.Bacc("TRN2", target_bir_lowering=False, debug=False, num_devices=NCORES)

    # host-pretiled: [ki, bt, ko, bi]
    x1p = nc.declare_dram_parameter("x1p", [P, BT, KT, P], mmdt, isOutput=False)
    x2p = nc.declare_dram_parameter("x2p", [P, BT, KT, P], mmdt, isOutput=False)
    # host-pretiled: [ki, ko, o]
    a_d = nc.declare_dram_parameter("a", [P, KT, OUT], mmdt, isOutput=False)
    y_d = nc.declare_dram_parameter("y", [R, OUT], outdt, isOutput=True)

    with TileContext(nc) as tc:
        with (
            tc.tile_pool(name="singles", bufs=1) as singles,
            tc.tile_pool(name="xs", bufs=3) as xpool,
            tc.tile_pool(name="ns", bufs=3) as npool,
            tc.tile_pool(name="outs", bufs=3) as opool,
            tc.tile_pool(name="stats", bufs=4) as stats,
            tc.tile_pool(name="psum", bufs=2, space="PSUM") as psum,
        ):
            # --- PE warmup on a memset tile: no DMA dependency, so Tensor
            # activity (and the HAM/clock ramp) starts right at 'main'.
            # Alternating two stationaries lets each LDWEIGHTS overlap the
            # previous matmul (background weight buffer).
            warm_sb = singles.tile([P, 512], bf16)
            nc.vector.memset(warm_sb, 0.5)
            warm_ps = psum.tile([P, 512], f32, tag="ps11")
            for w in range(N_WARMUP):
                lo = 128 * (w % 2)
                nc.tensor.matmul(
                    warm_ps[:], lhsT=warm_sb[:, lo : lo + P], rhs=warm_sb[:],
                    start=True, stop=True,
                )

            # --- startup race: bt=0 x tiles on the GpSimd queue, A k-tiles
            # on the Vector queue, so descriptor issue (~0.6us each) is not
            # serialized behind one engine and the first matmul's deps land
            # as early as possible.
            xt_pre = {}
            for s, xp in enumerate((x1p, x2p)):
                t = xpool.tile([P, KT, P], mmdt, tag=f"xt{s}", name=f"xt_pre_{s}")
                nc.sync.dma_start(t[:], xp[:, 0])
                xt_pre[(0, s)] = t

            a_sb = [None] * KT
            for k in range(KT):
                t = singles.tile([P, OUT], mmdt, tag=f"a{k}", name=f"a{k}")
                nc.gpsimd.dma_start(t[:], a_d[:, k, :])
                a_sb[k] = t

            eps_sb = singles.tile([P, 1], f32)
            nc.vector.memset(eps_sb, EPS)

            def epilogue(bt, s, ps_tiles):
                """LayerNorm the two PSUM halves of group (bt, s) -> SBUF."""
                st = stats.tile([P, NH, 6], f32, tag=f"st{s}", name=f"st{bt}{s}")
                for h in range(NH):
                    nc.vector.bn_stats(st[:, h, :], ps_tiles[h][:])
                mv = stats.tile([P, 2], f32, tag=f"mv{s}", name=f"mv{bt}{s}")
                nc.vector.bn_aggr(mv[:], st[:])

                r_sb = stats.tile([P, 1], f32, tag=f"r{s}", name=f"r{bt}{s}")
                nc.scalar.activation(
                    r_sb[:], mv[:, 1:2], func=AF.Sqrt, bias=eps_sb[:], scale=1.0
                )
                nc.vector.reciprocal(r_sb[:], r_sb[:])
                nmr = stats.tile([P, 1], f32, tag=f"nmr{s}", name=f"nmr{bt}{s}")
                nc.vector.tensor_scalar(
                    nmr[:],
                    mv[:, 0:1],
                    scalar1=r_sb[:],
                    scalar2=-1.0,
                    op0=ALU.mult,
                    op1=ALU.mult,
                )

                ntile = npool.tile([P, OUT], f32, tag=f"n{s}", name=f"n{bt}{s}")
                for h in range(NH):
                    nc.scalar.activation(
                        ntile[:, h * 512 : (h + 1) * 512],
                        ps_tiles[h][:],
                        func=AF.Identity,
                        bias=nmr[:],
                        scale=r_sb[:],
                    )
                return ntile

            def store(bt, n_pair):
                out_t = opool.tile([P, OUT], outdt, tag="out", name=f"out{bt}")
                for h in range(NH):
                    sl = slice(h * 512, (h + 1) * 512)
                    nc.vector.tensor_tensor(
                        out_t[:, sl], n_pair[0][:, sl], n_pair[1][:, sl],
                        op=ALU.add,
                    )
                nc.sync.dma_start(y_d[bt * P : (bt + 1) * P, :], out_t[:])

            for bt in range(BT):
                xts = {}
                for s in range(2):
                    if (bt, s) in xt_pre:
                        xts[s] = xt_pre[(bt, s)]
                    else:
                        xt = xpool.tile(
                            [P, KT, P], mmdt, tag=f"xt{s}", name=f"xt{bt}_{s}"
                        )
                        nc.sync.dma_start(xt[:], (x1p, x2p)[s][:, bt])
                        xts[s] = xt

                ps = {
                    s: [
                        psum.tile(
                            [P, 512], f32, tag=f"ps{s}{h}",
                            name=f"ps{bt}{s}{h}",
                        )
                        for h in range(NH)
                    ]
                    for s in range(2)
                }

                if bt < BT - 1:
                    # k-major across both streams: 4 matmuls per A k-tile,
                    # so the PE keeps pace with the A DMA stream at kernel
                    # start.
                    for k in range(KT):
                        for s in range(2):
                            for h in range(NH):
                                nc.tensor.matmul(
                                    ps[s][h][:],
                                    lhsT=xts[s][:, k, :],
                                    rhs=a_sb[k][:, h * 512 : (h + 1) * 512],
                                    start=(k == 0),
                                    stop=(k == KT - 1),
                                )
                    n_pair = [epilogue(bt, s, ps[s]) for s in range(2)]
                    store(bt, n_pair)
                else:
                    # Tail: serialize the streams so epilogues stagger; the
                    # very last stream runs h-outer so its h0 stats overlap
                    # its h1 matmuls.
                    n_pair = []
                    for s in range(2):
                        order = (
                            [(h, k) for h in range(NH) for k in range(KT)]
                            if s == 1
                            else [(h, k) for k in range(KT) for h in range(NH)]
                        )
                        for h, k in order:
                            nc.tensor.matmul(
                                ps[s][h][:],
                                lhsT=xts[s][:, k, :],
                                rhs=a_sb[k][:, h * 512 : (h + 1) * 512],
                                start=(k == 0),
                                stop=(k == KT - 1),
                            )
                        n_pair.append(epilogue(bt, s, ps[s]))
                    store(bt, n_pair)

    nc.finalize()
    return nc


def _get_nc(mm_dtype_name: str, out_dtype_name: str):
    key = (mm_dtype_name, out_dtype_name)
    if key not in _cache:
        _cache[key] = _build(mm_dtype_name, out_dtype_name)
    return _cache[key]


def _pretile_x(x_core: np.ndarray) -> np.ndarray:
    # [R, C] -> [ki, bt, ko, bi]
    return np.ascontiguousarray(
        x_core.reshape(BT, P, KT, P).transpose(3, 0, 2, 1)
    )


def kernel(x1, x2, W_Q, W_K, W_V, W_fc, gamma, beta, _trace=False,
           _mm_dtype="bfloat16", _out_dtype="bfloat16"):
    from concourse.bass_utils import run_bass_kernel_spmd

    x1 = np.asarray(x1, dtype=np.float32)
    x2 = np.asarray(x2, dtype=np.float32)
    W_V = np.asarray(W_V, dtype=np.float32)
    W_fc = np.asarray(W_fc, dtype=np.float32)
    gamma = np.asarray(gamma, dtype=np.float32)
    beta = np.asarray(beta, dtype=np.float32)

    # A = W_V.T @ W_fc.T in float64 to keep the host collapse error negligible.
    A = (W_V.T.astype(np.float64) @ W_fc.T.astype(np.float64)).astype(np.float32)
    # [C, OUT] -> [ki, ko, o]
    Ap = np.ascontiguousarray(A.reshape(KT, P, OUT).transpose(1, 0, 2))

    # Device computes LN(y1)+LN(y2); any affine LN params fold in on host:
    # out = (LN1+LN2)*gamma + 2*beta.  (This problem has gamma=1, beta=0.)
    use_affine = not (np.all(gamma == 1.0) and np.all(beta == 0.0))

    if _mm_dtype == "bfloat16":
        import ml_dtypes

        np_mm = ml_dtypes.bfloat16
    else:
        np_mm = np.float32
    Ap = Ap.astype(np_mm)

    in_maps = []
    for r in range(NCORES):
        sl = slice(r * R, (r + 1) * R)
        m = {
            "x1p": _pretile_x(x1[sl]).astype(np_mm),
            "x2p": _pretile_x(x2[sl]).astype(np_mm),
            "a": Ap,
        }
        in_maps.append(m)

    nc = _get_nc(_mm_dtype, _out_dtype)
    res = run_bass_kernel_spmd(nc, in_maps, list(range(NCORES)), trace=_trace)

    y = np.concatenate(
        [np.asarray(res.results[r]["y"], dtype=np.float32) for r in range(NCORES)],
        axis=0,
    )
    if use_affine:
        y = y * gamma[None, :] + 2.0 * beta[None, :]
    out = y.reshape(B, 1, OUT)
    if _trace:
        return out, res
    return out


# revision 7
# speedup vs baseline: 1.0833x; 1.0239x over previous
"""Trainium2 kernel for nn_MultiHeadCrossAttention_28063316313030.

Math: with seq_len == 1, softmax over a size-1 axis is identically 1, so
attention(Q,K,V) == V and W_Q/W_K are dead code.  The whole module collapses to

    out = LN(x1 @ A) + LN(x2 @ A),   A = W_V.T @ W_fc.T   (1024 x 1024)

where LN is LayerNorm over the last dim (gamma/beta fold in on host).

Distribution: pure data parallel over the batch dim across 8 NeuronCores.
Host precomputes A (tiny matmul) and pre-tiles x1/x2 C-major so the TensorE
contraction dim lands on SBUF partitions with fully contiguous DMA runs.

Everything on the PE path is bf16 (x, A, and the stored output, which the
host upcasts to f32): the moving operand streams at ~218ns per 512-row
matmul vs ~233ns for fp32r, and DMA bytes halve.  LayerNorm stays in f32
(PSUM + stats).  Measured rel err ~4e-3 vs the 2e-2 gate.

Device per core (2048 rows per stream, 16 b-tiles x 2 streams):
  PE warmup matmuls run on a memset tile (no DMA dependency) so Tensor
  activity (clock ramp) starts right after sequencer 'main'.
  DMA issue engines: A k-tiles split gpsimd (even k) / scalar (odd k) so
  the 8 descriptors issue in parallel; x tiles on sync (which never blocks:
  stores live on gpsimd, so x prefetch stays 2 tiles deep); outputs on
  gpsimd, except the very last b-tile which splits its store per-512-half
  across gpsimd+sync to overlap the tail DMA.
  b-tiles are processed k-major across the 4 (stream, half) PSUM banks:
  each A k-tile feeds 4 matmuls, so the PE keeps pace with the A stream
  at kernel start.
  Fused epilogue per b-tile: bn_stats/bn_aggr per stream -> mean/var,
  r = Rsqrt(var+eps) (one ACT op), nmr = -mu*r (DVE).  Stream-0 normalizes
  via ACT Identity with bias = nmr0+nmr1; stream-1 then fuses normalize
  and the cross-stream add in ONE DVE pass:
      out = (ps1 * r1) + n0'   (scalar_tensor_tensor, bf16 out)
  which writes the final bf16 tile DMA'd to HBM.  The last b-tile instead
  keeps n0 = ps0*r0 + nmr0 off the critical path (computed during the s1
  matmuls) and finishes with tensor_scalar + tensor_tensor per half.
"""

import sys

sys.path.insert(0, "/opt/trn_rl_repo")

import numpy as np

B, C, OUT = 16384, 1024, 1024
EPS = 1e-5
NCORES = 8
R = B // NCORES  # rows per core per stream
P = 128
KT = C // P  # contraction tiles
BT = R // P  # row tiles per core
NH = OUT // 512  # psum bank halves per row tile
N_WARMUP = 5

_cache = {}


def _build(mm_dtype_name: str, out_dtype_name: str):
    import concourse.bacc as bacc
    import concourse.bass as bass
    import concourse.mybir as mybir
    from concourse.tile import TileContext

    f32 = mybir.dt.float32
    bf16 = mybir.dt.bfloat16
    mmdt = getattr(mybir.dt, mm_dtype_name)
    outdt = getattr(mybir.dt, out_dtype_name)
    AF = mybir.ActivationFunctionType
    ALU = mybir.AluOpType

    nc = bacc.Bacc("TRN2", target_bir_lowering=False, debug=False, num_devices=NCORES)

    # host-pretiled: [ki, bt, ko, bi]
    x1p = nc.declare_dram_parameter("x1p", [P, BT, KT, P], mmdt, isOutput=False)
    x2p = nc.declare_dram_parameter("x2p", [P, BT, KT, P], mmdt, isOutput=False)
    # host-pretiled: [ki, ko, o]
    a_d = nc.declare_dram_parameter("a", [P, KT, OUT], mmdt, isOutput=False)
    y_d = nc.declare_dram_parameter("y", [R, OUT], outdt, isOutput=True)

    with TileContext(nc) as tc:
        with (
            tc.tile_pool(name="singles", bufs=1) as singles,
            tc.tile_pool(name="xs", bufs=4) as xpool,
            tc.tile_pool(name="ns", bufs=3) as npool,
            tc.tile_pool(name="outs", bufs=3) as opool,
            tc.tile_pool(name="stats", bufs=4) as stats,
            tc.tile_pool(name="psum", bufs=2, space="PSUM") as psum,
        ):
            # --- PE warmup on a memset tile: no DMA dependency.
            warm_sb = singles.tile([P, 512], bf16)
            nc.vector.memset(warm_sb, 0.5)
            warm_ps = psum.tile([P, 512], f32, tag="ps11")
            for w in range(N_WARMUP):
                lo = 128 * (w % 2)
                nc.tensor.matmul(
                    warm_ps[:], lhsT=warm_sb[:, lo : lo + P], rhs=warm_sb[:],
                    start=True, stop=True,
                )

            # --- startup race: x b-tiles 0/1 on Sync, A k-tiles alternating
            # GpSimd / Scalar so descriptor issue (~0.6us each) parallelizes.
            xt_pre = {}
            for bt0 in range(2):
                for s, xp in enumerate((x1p, x2p)):
                    t = xpool.tile(
                        [P, KT, P], mmdt, tag=f"xt{s}", name=f"xt_pre{bt0}_{s}"
                    )
                    nc.sync.dma_start(t[:], xp[:, bt0])
                    xt_pre[(bt0, s)] = t

            a_sb = [None] * KT
            for k in range(KT):
                t = singles.tile([P, OUT], mmdt, tag=f"a{k}", name=f"a{k}")
                eng = nc.gpsimd if k % 2 == 0 else nc.scalar
                eng.dma_start(t[:], a_d[:, k, :])
                a_sb[k] = t

            eps_sb = singles.tile([P, 1], f32)
            nc.vector.memset(eps_sb, EPS)

            def stream_stats(bt, s, ps_tiles):
                """bn stats -> r = rsqrt(var+eps), nmr = -mu*r for one stream."""
                st = stats.tile([P, NH, 6], f32, tag=f"st{s}", name=f"st{bt}{s}")
                for h in range(NH):
                    nc.vector.bn_stats(st[:, h, :], ps_tiles[h][:])
                mv = stats.tile([P, 2], f32, tag=f"mv{s}", name=f"mv{bt}{s}")
                nc.vector.bn_aggr(mv[:], st[:])
                r_sb = stats.tile([P, 1], f32, tag=f"r{s}", name=f"r{bt}{s}")
                nc.scalar.activation(
                    r_sb[:], mv[:, 1:2], func=AF.Sqrt, bias=eps_sb[:], scale=1.0
                )
                nc.vector.reciprocal(r_sb[:], r_sb[:])
                nmr = stats.tile([P, 1], f32, tag=f"nmr{s}", name=f"nmr{bt}{s}")
                nc.vector.tensor_scalar(
                    nmr[:],
                    mv[:, 0:1],
                    scalar1=r_sb[:],
                    scalar2=-1.0,
                    op0=ALU.mult,
                    op1=ALU.mult,
                )
                return r_sb, nmr

            for bt in range(BT):
                xts = {}
                for s in range(2):
                    if (bt, s) in xt_pre:
                        xts[s] = xt_pre[(bt, s)]
                    else:
                        xts[s] = xt_pre[(bt, s)]  # unreachable; kept for clarity
                # prefetch b-tile bt+2
                if bt + 2 < BT:
                    for s in range(2):
                        t = xpool.tile(
                            [P, KT, P], mmdt, tag=f"xt{s}", name=f"xt{bt + 2}_{s}"
                        )
                        nc.sync.dma_start(t[:], (x1p, x2p)[s][:, bt + 2])
                        xt_pre[(bt + 2, s)] = t

                ps = {
                    s: [
                        psum.tile(
                            [P, 512], f32, tag=f"ps{s}{h}", name=f"ps{bt}{s}{h}"
                        )
                        for h in range(NH)
                    ]
                    for s in range(2)
                }

                if bt < BT - 1:
                    # k-major across both streams: 4 matmuls per A k-tile.
                    for k in range(KT):
                        for s in range(2):
                            for h in range(NH):
                                nc.tensor.matmul(
                                    ps[s][h][:],
                                    lhsT=xts[s][:, k, :],
                                    rhs=a_sb[k][:, h * 512 : (h + 1) * 512],
                                    start=(k == 0),
                                    stop=(k == KT - 1),
                                )
                    r0, nmr0 = stream_stats(bt, 0, ps[0])
                    r1, nmr1 = stream_stats(bt, 1, ps[1])
                    nmrs = stats.tile([P, 1], f32, tag="nmrs", name=f"nmrs{bt}")
                    nc.vector.tensor_tensor(nmrs[:], nmr0[:], nmr1[:], op=ALU.add)
                    ntile = npool.tile([P, OUT], f32, tag="n0", name=f"n{bt}")
                    out_t = opool.tile([P, OUT], outdt, tag="out", name=f"out{bt}")
                    for h in range(NH):
                        sl = slice(h * 512, (h + 1) * 512)
                        nc.scalar.activation(
                            ntile[:, sl], ps[0][h][:],
                            func=AF.Identity, bias=nmrs[:], scale=r0[:],
                        )
                        nc.vector.scalar_tensor_tensor(
                            out_t[:, sl], ps[1][h][:], r1[:], ntile[:, sl],
                            op0=ALU.mult, op1=ALU.add,
                        )
                    nc.gpsimd.dma_start(y_d[bt * P : (bt + 1) * P, :], out_t[:])
                else:
                    # Tail b-tile: streams serialized; s0's normalize runs
                    # during s1's matmuls; s1 h-outer so h0 stats overlap
                    # h1 matmuls; store split per half on two engines.
                    for k in range(KT):
                        for h in range(NH):
                            nc.tensor.matmul(
                                ps[0][h][:],
                                lhsT=xts[0][:, k, :],
                                rhs=a_sb[k][:, h * 512 : (h + 1) * 512],
                                start=(k == 0),
                                stop=(k == KT - 1),
                            )
                    r0, nmr0 = stream_stats(bt, 0, ps[0])
                    ntile = npool.tile([P, OUT], f32, tag="n0", name=f"n{bt}")
                    for h in range(NH):
                        sl = slice(h * 512, (h + 1) * 512)
                        nc.scalar.activation(
                            ntile[:, sl], ps[0][h][:],
                            func=AF.Identity, bias=nmr0[:], scale=r0[:],
                        )
                    for h in range(NH):
                        for k in range(KT):
                            nc.tensor.matmul(
                                ps[1][h][:],
                                lhsT=xts[1][:, k, :],
                                rhs=a_sb[k][:, h * 512 : (h + 1) * 512],
                                start=(k == 0),
                                stop=(k == KT - 1),
                            )
                    r1, nmr1 = stream_stats(bt, 1, ps[1])
                    out_t = opool.tile([P, OUT], outdt, tag="out", name=f"out{bt}")
                    for h in range(NH):
                        sl = slice(h * 512, (h + 1) * 512)
                        th = npool.tile([P, 512], f32, tag=f"tt{h}", name=f"t{h}")
                        nc.vector.tensor_scalar(
                            th[:], ps[1][h][:],
                            scalar1=r1[:], scalar2=nmr1[:],
                            op0=ALU.mult, op1=ALU.add,
                        )
                        nc.vector.tensor_tensor(
                            out_t[:, sl], th[:], ntile[:, sl], op=ALU.add
                        )
                        eng = nc.gpsimd if h == 0 else nc.sync
                        eng.dma_start(
                            y_d[bt * P : (bt + 1) * P, h * 512 : (h + 1) * 512],
                            out_t[:, sl],
                        )

    nc.finalize()
    return nc


def _get_nc(mm_dtype_name: str, out_dtype_name: str):
    key = (mm_dtype_name, out_dtype_name)
    if key not in _cache:
        _cache[key] = _build(mm_dtype_name, out_dtype_name)
    return _cache[key]


def _pretile_x(x_core: np.ndarray) -> np.ndarray:
    # [R, C] -> [ki, bt, ko, bi]
    return np.ascontiguousarray(
        x_core.reshape(BT, P, KT, P).transpose(3, 0, 2, 1)
    )


def kernel(x1, x2, W_Q, W_K, W_V, W_fc, gamma, beta, _trace=False,
           _mm_dtype="bfloat16", _out_dtype="bfloat16"):
    from concourse.bass_utils import run_bass_kernel_spmd

    x1 = np.asarray(x1, dtype=np.float32)
    x2 = np.asarray(x2, dtype=np.float32)
    W_V = np.asarray(W_V, dtype=np.float32)
    W_fc = np.asarray(W_fc, dtype=np.float32)
    gamma = np.asarray(gamma, dtype=np.float32)
    beta = np.asarray(beta, dtype=np.float32)

    # A = W_V.T @ W_fc.T in float64 to keep the host collapse error negligible.
    A = (W_V.T.astype(np.float64) @ W_fc.T.astype(np.float64)).astype(np.float32)
    # [C, OUT] -> [ki, ko, o]
    Ap = np.ascontiguousarray(A.reshape(KT, P, OUT).transpose(1, 0, 2))

    # Device computes LN(y1)+LN(y2); any affine LN params fold in on host:
    # out = (LN1+LN2)*gamma + 2*beta.  (This problem has gamma=1, beta=0.)
    use_affine = not (np.all(gamma == 1.0) and np.all(beta == 0.0))

    if _mm_dtype == "bfloat16":
        import ml_dtypes

        np_mm = ml_dtypes.bfloat16
    else:
        np_mm = np.float32
    Ap = Ap.astype(np_mm)

    in_maps = []
    for r in range(NCORES):
        sl = slice(r * R, (r + 1) * R)
        m = {
            "x1p": _pretile_x(x1[sl]).astype(np_mm),
            "x2p": _pretile_x(x2[sl]).astype(np_mm),
            "a": Ap,
        }
        in_maps.append(m)

    nc = _get_nc(_mm_dtype, _out_dtype)
    res = run_bass_kernel_spmd(nc, in_maps, list(range(NCORES)), trace=_trace)

    y = np.concatenate(
        [np.asarray(res.results[r]["y"], dtype=np.float32) for r in range(NCORES)],
        axis=0,
    )
    if use_affine:
        y = y * gamma[None, :] + 2.0 * beta[None, :]
    out = y.reshape(B, 1, OUT)
    if _trace:
        return out, res
    return out
